# revision 26
# baseline (speedup 1.0000x reference)
"""FRQI encoding kernel for Trainium2 (8 NeuronCores, data-parallel).

Closed form of the reference: for each sample b with 4 pixels x[b, 0:4],
  out[b] = [0.0, 0.0, mean_i cos(x[b, i] * pi / 255)]
The two address-qubit columns are input-independent and exactly zero
(mean over 4 pixel indices of (-1)^bit is 0 for both address bits).

Device kernel (per core, 524288 samples = 2097152 input floats):
  - tiles of (128 partitions x F floats), contiguous DMA in; non-uniform
    F schedule (large tiles first, small last) to shrink the tail latency
    of last-tile compute + store
  - ScalarE activation Sin(x * pi/255 - pi/2) == -cos(2*theta), in-place
    (the HW Sin spline is only accurate on ~[-pi, pi]; -pi/2 bias keeps
    arguments in [-pi/2, pi/2) and the sign folds into the mean scale)
  - VectorE reduce_sum over innermost groups of 4
  - VectorE tensor_scalar_mul by -0.25 into the strided color column
    (offset 2, stride 3) of a per-tile persistent output buffer whose
    zero columns are memset once on the (otherwise idle) GpSimd engine,
    overlapped with the input DMAs
  - contiguous DMA out of the interleaved (128, 3*F/4) output tile
"""

import math
import sys

for _p in ("/opt/trn_rl_repo",):
    if _p not in sys.path:
        sys.path.append(_p)

import numpy as np

import concourse.bass as bass
import concourse.mybir as mybir
from concourse import bacc
from concourse.bass_utils import run_bass_kernel_spmd
from concourse.tile import TileContext

N_CORES = 8
B = 4_194_304
N_PIX = 4
N_PER_CORE = B // N_CORES          # 524288 samples
P = 128                            # SBUF partitions
L = N_PER_CORE * N_PIX             # 2097152 input floats per core
LO = N_PER_CORE * 3                # 1572864 output floats per core

# Per-tile free-dim sizes (floats per partition). Front-loaded large for
# DMA efficiency, small at the end to minimize tail latency.
F_SCHED = [4096, 4096, 4096, 2048, 1024, 1024]
assert sum(F_SCHED) * P == L

_SCALE = math.pi / 255.0           # x * pi/255 = 2 * theta
# cos(z) = -sin(z - pi/2); z - pi/2 lies in [-pi/2, pi/2), the accurate
# domain of the HW Sin spline (it degrades badly beyond ~pi). The sign
# flip is folded into the final mean scale (-0.25).
_BIAS = -math.pi / 2.0


def _build_nc() -> bass.Bass:
    # Bacc (not raw Bass): its compile() pass generate_event_semaphores
    # splits multi-sem waits to satisfy the 1-wait-per-instruction HW limit.
    nc = bacc.Bacc()
    f32 = mybir.dt.float32
    x = nc.dram_tensor("x", [L], f32, kind="ExternalInput")
    y = nc.dram_tensor("y", [LO], f32, kind="ExternalOutput")

    # Persistent per-tile output buffers; zeroed inside the TileContext
    # (below) so the memsets overlap the input DMAs.
    bias_t = nc.alloc_sbuf_tensor("bias_mpi2", [P, 1], f32)
    obufs = [
        nc.alloc_sbuf_tensor(f"ob{t}", [P, 3 * (f // N_PIX)], f32)
        for t, f in enumerate(F_SCHED)
    ]
    bias_ap = bias_t.ap()

    with TileContext(nc) as tc:
        # One slot per uniquely-tagged tile: no slot reuse, so no in-DMA
        # ever carries a WAR wait and the Sync sequencer can dispatch
        # every input DMA up front; slots are sized per tile (a shared
        # tag would size every slot to the largest tile).
        with tc.tile_pool(name="io", bufs=1) as pool:
            nc.gpsimd.memset(bias_ap, _BIAS)
            # Zero the address-qubit columns on the idle GpSimd engine,
            # overlapped with the first input DMAs. The loop only ever
            # rewrites the color column (2::3).
            for ob in obufs:
                nc.gpsimd.memset(ob[:], 0.0)
            in_off = 0
            out_off = 0
            stores = []
            for t, F in enumerate(F_SCHED):
                C = F // N_PIX
                x_t = x[in_off:in_off + P * F].rearrange("(p f) -> p f", p=P)
                y_t = y[out_off:out_off + P * 3 * C].rearrange(
                    "(p f) -> p f", p=P
                )
                it = pool.tile([P, F], f32, tag=f"in{t}")
                nc.sync.dma_start(out=it[:], in_=x_t)
                nc.scalar.activation(
                    it[:], it[:], mybir.ActivationFunctionType.Sin,
                    bias=bias_ap, scale=_SCALE,
                )
                st = pool.tile([P, C], f32, tag=f"sum{t}")
                nc.vector.reduce_sum(
                    st[:],
                    it[:].rearrange("p (c r) -> p c r", r=N_PIX),
                    axis=mybir.AxisListType.X,
                )
                ob = obufs[t]
                nc.vector.tensor_scalar_mul(ob[:, 2:3 * C:3], st[:], -0.25)
                stores.append((y_t, ob))
                in_off += P * F
                out_off += P * 3 * C
            # All output DMAs after every input DMA in the Sync engine's
            # program order: the sequencer blocks on the first store's
            # wait only after all input DMAs are dispatched, and the
            # FIFO ring then drains inputs before outputs (inputs gate
            # compute; outputs are fire-and-forget).
            for y_t, ob in stores:
                nc.scalar.dma_start(out=y_t, in_=ob[:])
    nc.finalize()
    return nc


_NC_CACHE = None


def _get_nc() -> bass.Bass:
    global _NC_CACHE
    if _NC_CACHE is None:
        _NC_CACHE = _build_nc()
    return _NC_CACHE


def _run(x: np.ndarray, **spmd_kwargs):
    """x: (B, 4) float32. Returns (full_output, BassKernelResults)."""
    shards = x.reshape(N_CORES, L)
    in_maps = [{"x": shards[i]} for i in range(N_CORES)]
    res = run_bass_kernel_spmd(_get_nc(), in_maps, list(range(N_CORES)), **spmd_kwargs)
    out = np.concatenate(
        [r["y"].reshape(N_PER_CORE, 3) for r in res.results], axis=0
    )
    return out, res


def kernel(**inputs: np.ndarray) -> np.ndarray:
    x = np.ascontiguousarray(
        np.asarray(inputs["inputs"], dtype=np.float32)
    ).reshape(B, N_PIX)
    out, _ = _run(x)
    return out


# revision 27
# speedup vs baseline: 1.1560x; 1.1560x over previous
"""FRQI encoding kernel for Trainium2 (8 NeuronCores, data-parallel).

Closed form of the reference: for each sample b with 4 pixels x[b, 0:4],
  out[b] = [0.0, 0.0, mean_i cos(x[b, i] * pi / 255)]
The two address-qubit columns are input-independent and exactly zero
(mean over 4 pixel indices of (-1)^bit is 0 for both address bits).

Device kernel (per core, 524288 samples = 2097152 input floats):
  - tiles of (128 partitions x F floats), contiguous DMA in; non-uniform
    F schedule (large tiles first, small last) to shrink the tail latency
    of last-tile compute + store
  - ScalarE activation Sin(x * pi/255 - pi/2) == -cos(2*theta), in-place
    (the HW Sin spline is only accurate on ~[-pi, pi]; -pi/2 bias keeps
    arguments in [-pi/2, pi/2) and the sign folds into the mean scale)
  - VectorE reduce_sum over innermost groups of 4
  - VectorE tensor_scalar_mul by -0.25 into the strided color column
    (offset 2, stride 3) of a per-tile persistent output buffer whose
    zero columns are memset once on the (otherwise idle) GpSimd engine,
    overlapped with the input DMAs
  - contiguous DMA out of the interleaved (128, 3*F/4) output tile
"""

import math
import sys

for _p in ("/opt/trn_rl_repo",):
    if _p not in sys.path:
        sys.path.append(_p)

import numpy as np

import concourse.bass as bass
import concourse.mybir as mybir
from concourse import bacc
from concourse.bass_utils import run_bass_kernel_spmd
from concourse.tile import TileContext

N_CORES = 8
B = 4_194_304
N_PIX = 4
N_PER_CORE = B // N_CORES          # 524288 samples
P = 128                            # SBUF partitions
L = N_PER_CORE * N_PIX             # 2097152 input floats per core
LO = N_PER_CORE * 3                # 1572864 output floats per core

# Per-tile free-dim sizes (floats per partition). Front-loaded large for
# DMA efficiency, small at the end to minimize tail latency.
F_SCHED = [4096, 4096, 4096, 2048, 1024, 1024]
assert sum(F_SCHED) * P == L

_SCALE = math.pi / 255.0           # x * pi/255 = 2 * theta
# cos(z) = -sin(z - pi/2); z - pi/2 lies in [-pi/2, pi/2), the accurate
# domain of the HW Sin spline (it degrades badly beyond ~pi). The sign
# flip is folded into the final mean scale (-0.25).
_BIAS = -math.pi / 2.0


def _build_nc() -> bass.Bass:
    # Bacc (not raw Bass): its compile() pass generate_event_semaphores
    # splits multi-sem waits to satisfy the 1-wait-per-instruction HW limit.
    nc = bacc.Bacc()
    f32 = mybir.dt.float32
    x = nc.dram_tensor("x", [L], f32, kind="ExternalInput")
    y = nc.dram_tensor("y", [LO], f32, kind="ExternalOutput")

    # Persistent per-tile output buffers; zeroed inside the TileContext
    # (below) so the memsets overlap the input DMAs.
    bias_t = nc.alloc_sbuf_tensor("bias_mpi2", [P, 1], f32)
    obufs = [
        nc.alloc_sbuf_tensor(f"ob{t}", [P, 3 * (f // N_PIX)], f32)
        for t, f in enumerate(F_SCHED)
    ]
    bias_ap = bias_t.ap()

    with TileContext(nc) as tc:
        # One slot per uniquely-tagged tile: no slot reuse, so no in-DMA
        # ever carries a WAR wait and the Sync sequencer can dispatch
        # every input DMA up front; slots are sized per tile (a shared
        # tag would size every slot to the largest tile).
        with tc.tile_pool(name="io", bufs=1) as pool:
            nc.gpsimd.memset(bias_ap, _BIAS)
            # Zero the address-qubit columns on the idle GpSimd engine,
            # overlapped with the first input DMAs. The loop only ever
            # rewrites the color column (2::3).
            for ob in obufs:
                nc.gpsimd.memset(ob[:], 0.0)
            in_off = 0
            out_off = 0
            stores = []
            for t, F in enumerate(F_SCHED):
                C = F // N_PIX
                x_t = x[in_off:in_off + P * F].rearrange("(p f) -> p f", p=P)
                y_t = y[out_off:out_off + P * 3 * C].rearrange(
                    "(p f) -> p f", p=P
                )
                it = pool.tile([P, F], f32, tag=f"in{t}")
                nc.sync.dma_start(out=it[:], in_=x_t)
                nc.scalar.activation(
                    it[:], it[:], mybir.ActivationFunctionType.Sin,
                    bias=bias_ap, scale=_SCALE,
                )
                st = pool.tile([P, C], f32, tag=f"sum{t}")
                nc.vector.reduce_sum(
                    st[:],
                    it[:].rearrange("p (c r) -> p c r", r=N_PIX),
                    axis=mybir.AxisListType.X,
                )
                ob = obufs[t]
                nc.vector.tensor_scalar_mul(ob[:, 2:3 * C:3], st[:], -0.25)
                stores.append((y_t, ob))
                in_off += P * F
                out_off += P * 3 * C
            # All output DMAs after every input DMA in the Sync engine's
            # program order: the sequencer blocks on the first store's
            # wait only after all input DMAs are dispatched, and the
            # FIFO ring then drains inputs before outputs (inputs gate
            # compute; outputs are fire-and-forget).
            for y_t, ob in stores:
                nc.sync.dma_start(out=y_t, in_=ob[:])
    nc.finalize()
    return nc


_NC_CACHE = None


def _get_nc() -> bass.Bass:
    global _NC_CACHE
    if _NC_CACHE is None:
        _NC_CACHE = _build_nc()
    return _NC_CACHE


def _run(x: np.ndarray, **spmd_kwargs):
    """x: (B, 4) float32. Returns (full_output, BassKernelResults)."""
    shards = x.reshape(N_CORES, L)
    in_maps = [{"x": shards[i]} for i in range(N_CORES)]
    res = run_bass_kernel_spmd(_get_nc(), in_maps, list(range(N_CORES)), **spmd_kwargs)
    out = np.concatenate(
        [r["y"].reshape(N_PER_CORE, 3) for r in res.results], axis=0
    )
    return out, res


def kernel(**inputs: np.ndarray) -> np.ndarray:
    x = np.ascontiguousarray(
        np.asarray(inputs["inputs"], dtype=np.float32)
    ).reshape(B, N_PIX)
    out, _ = _run(x)
    return out
